# revision 5
# baseline (speedup 1.0000x reference)
"""Locally-connected layer (unshared 3x3 conv, torch-unfold semantics) on 8 trn2 cores.

out[b,o,y,x] = sum_{c,i,j} weight[o, c*9+i*3+j, y*32+x] * xpad[b, c, y+i, x+j] + bias[o, l]

Sharding: spatial over L - core r owns image rows [4r, 4r+4) (128 pixels).

v2 design (vs v1 @ 385 us: DMA was 94% active at 10% bandwidth utilization
because weight DMAs were 128-byte packets):
  - everything bf16 on the wire (tolerance is 2e-2; bf16 matmul error ~1e-3).
  - weights are relaid out on the HOST into the exact SBUF stream layout, so
    every DMA is per-partition contiguous (6 KB packets instead of 128 B).
  - x slab is host-fused: partitions 64:128 hold the +1-column-shifted slab so
    kernel columns j=0,1 fuse into K=128 matmuls; j=2 runs as K=64 matmuls.
  - per output column x (32 per core): one PSUM tile [B=64, 4y*64o=256] bank
    accumulates 6 fused + 6 single matmuls (N = vi*64 <= 192 each), then one
    DVE copy -> bf16 staging tile -> DMA out every 4 columns.
  - weight stream chunked by 4 output columns (1.2 MB/chunk) on the sync DMA
    ring, double-buffered (bufs=3); outputs go on the scalar ring.

Per-core HBM traffic: w 9.44 MB + x 3.34 MB + out 1.05 MB = 13.8 MB bf16
(~38 us at 358 GB/s) vs 24.3 MB fp32 before.
"""

import numpy as np

B, C, O, H, W, KS = 64, 64, 64, 32, 32, 3
L = H * W
NCORES = 8
RPC = H // NCORES            # image rows per core = 4
SLABR = RPC + 2              # slab rows per core (with halo) = 6
XS = W + 2                   # padded slab width = 34

# per-slab-row block tables: output rows y served by slab row rp are
# y = rp - i for i in 0..2 clipped to [0, RPC)
YS = [max(0, rp - 2) for rp in range(SLABR)]           # first y
VI = [min(RPC - 1, rp) - max(0, rp - 2) + 1 for rp in range(SLABR)]  # n blocks
OFF = np.cumsum([0] + [v * O for v in VI]).tolist()    # col offset in 768
WCOLS = OFF[SLABR]                                     # 768 weight cols per x

_CACHE = {}


def _build_nc():
    import concourse.bass as bass
    import concourse.bacc as bacc
    import concourse.tile as tile
    from concourse import mybir

    f32 = mybir.dt.float32
    bf16 = mybir.dt.bfloat16
    nc = bacc.Bacc(
        "TRN2", target_bir_lowering=False, debug=False, num_devices=NCORES
    )
    # x slab, host-prefused: [p, xs, rp, b]; p<64 -> slab col xs, p>=64 -> xs+1
    x_d = nc.dram_tensor("xf", [128, XS, SLABR, B], bf16, kind="ExternalInput")
    # fused weight stream (j=0 lower, j=1 upper): [p=(j,c), x, col]
    wf_d = nc.dram_tensor("wf", [128, W, WCOLS], bf16, kind="ExternalInput")
    # j=2 weight stream: [c, x, col]
    ws_d = nc.dram_tensor("ws", [64, W, WCOLS], bf16, kind="ExternalInput")
    # out [b, x, (y, o)]
    o_d = nc.dram_tensor("out", [B, W, RPC * O], bf16, kind="ExternalOutput")

    # output-column chunks: small first chunks minimize the head latency
    # before the first matmul; later chunks are wide for big DMA packets
    XCH = [(0, 2), (2, 4), (4, 8), (8, 16), (16, 24), (24, 32)]
    # x-slab chunk needed for out chunk [a, b) is xs in [a, b+2)
    XFCH = [(0, 4), (4, 6), (6, 10), (10, 18), (18, 26), (26, 34)]

    with tile.TileContext(nc) as tc:
        with (
            tc.tile_pool(name="xp", bufs=1) as xpool,
            tc.tile_pool(name="wf", bufs=3) as wfpool,
            tc.tile_pool(name="ws", bufs=3) as wspool,
            tc.tile_pool(name="ot", bufs=2) as opool,
            tc.tile_pool(name="ps", bufs=4, space=bass.MemorySpace.PSUM) as pspool,
        ):
            xf = xpool.tile([128, XS, SLABR, B], bf16)
            for (a, b), (fa, fb) in zip(XCH, XFCH):
                # consumption-order interleave on the sync ring
                nc.sync.dma_start(xf[:, fa:fb], x_d[:, fa:fb])
                nch = b - a
                wfk = wfpool.tile([128, nch, WCOLS], bf16, name=f"wfk{a}", tag="wfk")
                nc.sync.dma_start(wfk[:], wf_d[:, a:b])
                wsk = wspool.tile([64, nch, WCOLS], bf16, name=f"wsk{a}", tag="wsk")
                nc.sync.dma_start(wsk[:], ws_d[:, a:b])
                ot = opool.tile([B, nch, RPC * O], bf16, name=f"ot{a}", tag="ot")

                for x in range(a, b):
                    xi = x - a
                    ps = pspool.tile([B, RPC * O], f32)
                    # fused j=0/1: K=128 (lower = slab col x, upper = x+1)
                    for rp in range(SLABR):
                        ya = YS[rp] * O
                        n = VI[rp] * O
                        nc.tensor.matmul(
                            ps[:, ya : ya + n],
                            xf[:, x, rp, :],
                            wfk[:, xi, OFF[rp] : OFF[rp] + n],
                            start=(rp == 0), stop=False,
                        )
                    # j=2: K=64, slab col x+2, lower half only
                    for rp in range(SLABR):
                        ya = YS[rp] * O
                        n = VI[rp] * O
                        nc.tensor.matmul(
                            ps[:, ya : ya + n],
                            xf[0:64, x + 2, rp, :],
                            wsk[:, xi, OFF[rp] : OFF[rp] + n],
                            start=False, stop=(rp == SLABR - 1),
                        )
                    nc.vector.tensor_copy(ot[:, xi, :], ps[:])
                nc.scalar.dma_start(o_d[:, a:b], ot[:])
    nc.compile()
    return nc


def _get_nc():
    if "nc" not in _CACHE:
        _CACHE["nc"] = _build_nc()
    return _CACHE["nc"]


def _shard_inputs(x, weight):
    from concourse import mybir

    bf16 = mybir.dt.np(mybir.dt.bfloat16)

    xpad = np.pad(x, ((0, 0), (0, 0), (1, 1), (1, 1)))  # (B, C, 34, 34)
    # XF[r, p, xs, rp, b]; lower half p=c, upper half = +1 column shift
    XF = np.zeros((NCORES, 128, XS, SLABR, B), np.float32)
    base = xpad.transpose(1, 3, 2, 0)  # (c, col, row, b)
    for rp in range(SLABR):
        # slab row rp of core r is padded row 4r+rp (8 cores)
        XF[:, 0:64, :, rp, :] = (
            base[:, :, rp : rp + 4 * NCORES : 4, :].transpose(2, 0, 1, 3)
        )
    XF[:, 64:128, 0 : XS - 1] = XF[:, 0:64, 1:XS]

    # weight streams; w6[o, c, i, j, y_img, x]
    w6 = weight.reshape(O, C, KS, KS, H, W)
    WF = np.zeros((NCORES, 128, W, WCOLS), np.float32)
    WS = np.zeros((NCORES, 64, W, WCOLS), np.float32)
    for rp in range(SLABR):
        for yi in range(VI[rp]):
            y = YS[rp] + yi
            i = rp - y
            csl = slice(OFF[rp] + yi * O, OFF[rp] + (yi + 1) * O)
            for j in (0, 1):
                # (O, C, R, W) -> (R, C, W, O)
                WF[:, j * 64 : (j + 1) * 64, :, csl] = (
                    w6[:, :, i, j, y::RPC, :].transpose(2, 1, 3, 0)
                )
            WS[:, :, :, csl] = w6[:, :, i, 2, y::RPC, :].transpose(2, 1, 3, 0)

    XF = XF.astype(bf16)
    WF = WF.astype(bf16)
    WS = WS.astype(bf16)
    return [
        {"xf": XF[r], "wf": WF[r], "ws": WS[r]} for r in range(NCORES)
    ]


def kernel(x, weight, bias, _trace=False, _trace_kwargs=None):
    from concourse.bass_utils import run_bass_kernel_spmd

    x = np.ascontiguousarray(np.asarray(x, dtype=np.float32))
    weight = np.asarray(weight, dtype=np.float32)
    bias = np.asarray(bias, dtype=np.float32)

    nc = _get_nc()
    in_maps = _shard_inputs(x, weight)
    res = run_bass_kernel_spmd(
        nc, in_maps, list(range(NCORES)),
        trace=_trace, **(_trace_kwargs or {}),
    )
    # per-core out [B, W, RPC*O] (b, x, y*64+o) -> (B, O, y, x)
    rows = [
        np.asarray(res.results[r]["out"], dtype=np.float32)
        .reshape(B, W, RPC, O)
        .transpose(0, 3, 2, 1)
        for r in range(NCORES)
    ]
    out = np.concatenate(rows, axis=2)  # (B, O, H, W)
    if np.any(bias):
        out = out + bias.reshape(1, O, H, W)
    if _trace:
        _CACHE["last_result"] = res
    return np.ascontiguousarray(out.astype(np.float32))
